# revision 2
# baseline (speedup 1.0000x reference)
"""AdaptiveTripletLoss on 8 TRN2 NeuronCores (Bass/Tile, SPMD).

Math: loss = mean over valid rows of relu(hp - hn + 0.5*(1+hp)) with
  hp = hardest (max) same-label distance, hn = hardest (min) other-label
  distance, distances on L2-normalized embeddings.

Device algorithm (per core, rows sharded):
  G' = En @ En.T - 8 * H @ H.T   (H = one-hot labels, 128 classes = full K)
  Same-label entries of G' sit in [-9,-7], different-label in [-1,1], so
    hn^2 = relu(2 - 2*max_j G')          (max over ALL columns)
    hp^2 = relu(-14 - 2*min_j G')        (min over a narrow window)
  Rows are sorted by label on the host and each core's column copy is
  rotated so its own 1024 rows sit at local columns [512, 1536); all
  same-label columns for a 128-row tile then fall in 3 static 512-chunks,
  so the -8*HH' correction and the min-reduce only touch those chunks.
"""

import sys

sys.path.insert(0, "/opt/trn_rl_repo")

import numpy as np

N_CORES = 8
B, D, NCLS = 8192, 128, 128
BC = B // N_CORES          # rows per core
ROLL = 512                 # own rows at local columns [ROLL, ROLL+BC)
NQ, QW = 4, 2048           # ET quarters
M_TILES = BC // 128        # 8 row tiles per core
G_GROUPS = 8               # 8 column groups of 1024
HTW_W = 2048               # one-hot window: local chunks 0..3

_cache = {}


def _build():
    import concourse.tile as tile
    from concourse import bacc, mybir

    f32 = mybir.dt.float32
    AX = mybir.AxisListType
    OP = mybir.AluOpType
    AF = mybir.ActivationFunctionType
    from concourse.bass import MemorySpace

    nc = bacc.Bacc("TRN2", target_bir_lowering=False, debug=False,
                   num_devices=N_CORES)
    emb_ext = nc.dram_tensor("emb", [B, D], f32, kind="ExternalInput")
    htw_ext = nc.dram_tensor("htw", [NCLS, HTW_W], f32, kind="ExternalInput")
    ident_ext = nc.dram_tensor("ident", [128, 128], f32, kind="ExternalInput")
    out_ext = nc.dram_tensor("out", [1, 2], f32, kind="ExternalOutput")

    with tile.TileContext(nc) as tc:
        with (
            tc.tile_pool(name="persist", bufs=1) as pp,
            tc.tile_pool(name="sq", bufs=2) as sq_pool,
            tc.tile_pool(name="diag", bufs=4) as diag_pool,
            tc.tile_pool(name="hneg", bufs=2) as hneg_pool,
            tc.tile_pool(name="fin", bufs=1) as fin_pool,
            tc.tile_pool(name="tp_ps", bufs=2, space=MemorySpace.PSUM) as tp_ps,
            tc.tile_pool(name="mm_ps", bufs=2, space=MemorySpace.PSUM) as mm_ps,
        ):
            # ---------- persistent SBUF ----------
            enat = [pp.tile([128, QW], f32, name=f"enat{q}", tag=f"enat{q}")
                    for q in range(NQ)]
            etq = [pp.tile([128, QW], f32, name=f"et{q}", tag=f"et{q}")
                   for q in range(NQ)]
            htw = pp.tile([NCLS, HTW_W], f32, name="htw_sb", tag="htw_sb")
            ident = pp.tile([128, 128], f32, name="ident_sb", tag="ident_sb")
            ssq = pp.tile([128, 64], f32, name="ssq", tag="ssq")
            rvec = pp.tile([128, 64], f32, name="rvec", tag="rvec")
            gmaxf = pp.tile([128, M_TILES * G_GROUPS], f32, name="gmaxf",
                            tag="gmaxf")
            gminw = pp.tile([128, M_TILES * 2], f32, name="gminw", tag="gminw")

            nc.gpsimd.dma_start(ident[:], ident_ext.ap())
            nc.gpsimd.dma_start(htw[:], htw_ext.ap())

            # ---------- prologue: load + row sum-of-squares ----------
            emb_ap = emb_ext.ap()
            for q in range(NQ):
                for t in range(16):
                    r0 = (q * 16 + t) * 128
                    nc.gpsimd.dma_start(enat[q][:, t * 128:(t + 1) * 128],
                                        emb_ap[r0:r0 + 128, :])
                sq = sq_pool.tile([128, QW], f32)
                nc.scalar.activation(sq[:], enat[q][:], AF.Square)
                nc.vector.tensor_reduce(
                    ssq[:, q * 16:(q + 1) * 16],
                    sq[:].rearrange("p (t d) -> p t d", d=128),
                    axis=AX.X, op=OP.add)

            # r = 1/||row|| = exp(-0.5*ln(ssq))
            lssq = fin_pool.tile([128, 64], f32, tag="lssq")
            nc.scalar.activation(lssq[:], ssq[:], AF.Ln)
            nc.scalar.activation(rvec[:], lssq[:], AF.Exp, scale=-0.5)

            # ---------- normalize + transpose:  ET[:,j] = E[j,:] * r_j ----------
            for q in range(NQ):
                for t in range(16):
                    gt = q * 16 + t
                    diag = diag_pool.tile([128, 128], f32, tag="diag")
                    nc.vector.tensor_scalar_mul(diag[:], ident[:],
                                                rvec[:, gt:gt + 1])
                    tp = tp_ps.tile([128, 128], f32, tag="tp")
                    nc.tensor.matmul(tp[:], enat[q][:, t * 128:(t + 1) * 128],
                                     diag[:], start=True, stop=True)
                    dst = etq[q][:, t * 128:(t + 1) * 128]
                    if t % 2 == 0:
                        nc.scalar.copy(dst, tp[:])
                    else:
                        nc.vector.tensor_copy(dst, tp[:])

            # ---------- main: G' tiles + row reduces ----------
            for m in range(M_TILES):
                c0 = ROLL + m * 128
                w = c0 // 512                      # 1 or 2
                win = (w - 1, w, w + 1)
                kxm_e = etq[0][:, c0:c0 + 128]
                hneg = hneg_pool.tile([128, 128], f32, tag="hneg")
                nc.vector.tensor_scalar_mul(hneg[:], htw[:, c0:c0 + 128], -8.0)

                for g in range(G_GROUPS):
                    ps = mm_ps.tile([128, 1024], f32, tag="mmg")
                    qq, qoff = g // 2, (g % 2) * 1024
                    in_win = [(2 * g + cc) in win for cc in range(2)]
                    for cc in range(2):
                        nc.tensor.matmul(
                            ps[:, cc * 512:(cc + 1) * 512], kxm_e,
                            etq[qq][:, qoff + cc * 512:qoff + (cc + 1) * 512],
                            start=True, stop=not in_win[cc])
                    for cc in range(2):
                        if in_win[cc]:
                            ch = 2 * g + cc
                            nc.tensor.matmul(
                                ps[:, cc * 512:(cc + 1) * 512], hneg[:],
                                htw[:, ch * 512:(ch + 1) * 512],
                                start=False, stop=True)
                    # neg side: max of G' over everything
                    nc.vector.tensor_reduce(
                        gmaxf[:, m * G_GROUPS + g:m * G_GROUPS + g + 1],
                        ps[:], axis=AX.X, op=OP.max)
                    # pos side: min of G' over the window slices
                    if g <= 1:
                        if w == 1:
                            lo, hi = (0, 1024) if g == 0 else (0, 512)
                        else:
                            lo, hi = (512, 1024) if g == 0 else (0, 1024)
                        nc.vector.tensor_reduce(
                            gminw[:, m * 2 + g:m * 2 + g + 1],
                            ps[:, lo:hi], axis=AX.X, op=OP.min)

            # ---------- finalize ----------
            gmax8 = fin_pool.tile([128, M_TILES], f32, tag="gmax8")
            nc.vector.tensor_reduce(
                gmax8[:], gmaxf[:].rearrange("p (m g) -> p m g", g=G_GROUPS),
                axis=AX.X, op=OP.max)
            gmin8 = fin_pool.tile([128, M_TILES], f32, tag="gmin8")
            nc.vector.tensor_reduce(
                gmin8[:], gminw[:].rearrange("p (m k) -> p m k", k=2),
                axis=AX.X, op=OP.min)

            hn2 = fin_pool.tile([128, M_TILES], f32, tag="hn2")
            nc.scalar.activation(hn2[:], gmax8[:], AF.Copy,
                                 scale=-2.0, bias=2.0)
            hn2m = fin_pool.tile([128, M_TILES], f32, tag="hn2m")
            nc.vector.tensor_scalar_max(hn2m[:], hn2[:], 1e-20)
            lhn = fin_pool.tile([128, M_TILES], f32, tag="lhn")
            nc.scalar.activation(lhn[:], hn2m[:], AF.Ln)
            hn = fin_pool.tile([128, M_TILES], f32, tag="hn")
            nc.scalar.activation(hn[:], lhn[:], AF.Exp, scale=0.5)

            hp2 = fin_pool.tile([128, M_TILES], f32, tag="hp2")
            nc.scalar.activation(hp2[:], gmin8[:], AF.Copy,
                                 scale=-2.0, bias=-14.0)
            hp2m = fin_pool.tile([128, M_TILES], f32, tag="hp2m")
            nc.vector.tensor_scalar_max(hp2m[:], hp2[:], 1e-20)
            lhp = fin_pool.tile([128, M_TILES], f32, tag="lhp")
            nc.scalar.activation(lhp[:], hp2m[:], AF.Ln)
            hp = fin_pool.tile([128, M_TILES], f32, tag="hp")
            nc.scalar.activation(hp[:], lhp[:], AF.Exp, scale=0.5)

            valid = fin_pool.tile([128, M_TILES], f32, tag="valid")
            nc.vector.tensor_scalar(valid[:], hn[:], 3.0, None, op0=OP.is_lt)

            t1 = fin_pool.tile([128, M_TILES], f32, tag="t1")
            nc.scalar.activation(t1[:], hp[:], AF.Copy, scale=1.5, bias=0.5)
            t2 = fin_pool.tile([128, M_TILES], f32, tag="t2")
            nc.vector.tensor_tensor(t2[:], t1[:], hn[:], op=OP.subtract)
            t3 = fin_pool.tile([128, M_TILES], f32, tag="t3")
            nc.vector.tensor_scalar_max(t3[:], t2[:], 0.0)
            t4 = fin_pool.tile([128, M_TILES], f32, tag="t4")
            nc.vector.tensor_tensor(t4[:], t3[:], valid[:], op=OP.mult)

            stacked = fin_pool.tile([128, 2], f32, tag="stacked")
            nc.vector.tensor_reduce(stacked[:, 0:1], t4[:], axis=AX.X, op=OP.add)
            nc.vector.tensor_reduce(stacked[:, 1:2], valid[:], axis=AX.X,
                                    op=OP.add)

            ones = fin_pool.tile([128, 1], f32, tag="ones")
            nc.gpsimd.memset(ones[:], 1.0)
            pfin = tp_ps.tile([1, 2], f32, tag="pfin", bufs=1)
            nc.tensor.matmul(pfin[:], ones[:], stacked[:], start=True, stop=True)
            outsb = fin_pool.tile([1, 2], f32, tag="outsb")
            nc.vector.tensor_copy(outsb[:], pfin[:])
            nc.gpsimd.dma_start(out_ext.ap(), outsb[:])

    nc.compile()
    return nc


def _get_nc():
    if "nc" not in _cache:
        _cache["nc"] = _build()
    return _cache["nc"]


def _prep_inputs(embeddings, labels):
    emb = np.ascontiguousarray(np.asarray(embeddings, dtype=np.float32))
    lab = np.asarray(labels).astype(np.int64).ravel()
    assert emb.shape == (B, D) and lab.shape == (B,)

    counts = np.bincount(lab, minlength=NCLS)
    present = counts[counts > 0]
    # window scheme needs bounded class extent; singleton classes would
    # change validity semantics. Both hold for this problem's data.
    assert present.max() < 384, f"class too large for window: {present.max()}"
    assert present.min() >= 2, "singleton class unsupported"

    perm = np.argsort(lab, kind="stable")
    emb_s = emb[perm]
    lab_s = lab[perm]

    ident = np.eye(128, dtype=np.float32)
    in_maps = []
    for c in range(N_CORES):
        shift = ROLL - BC * c
        emb_l = np.roll(emb_s, shift, axis=0)
        lab_l = np.roll(lab_s, shift)
        htw = (lab_l[None, :HTW_W] == np.arange(NCLS)[:, None]).astype(
            np.float32)
        in_maps.append({
            "emb": np.ascontiguousarray(emb_l),
            "htw": np.ascontiguousarray(htw),
            "ident": ident,
        })
    return in_maps


def kernel(embeddings, labels, _trace=False):
    from concourse.bass_utils import run_bass_kernel_spmd

    nc = _get_nc()
    in_maps = _prep_inputs(embeddings, labels)
    res = run_bass_kernel_spmd(nc, in_maps, core_ids=list(range(N_CORES)),
                               trace=_trace)
    total = 0.0
    count = 0.0
    for c in range(N_CORES):
        o = np.asarray(res.results[c]["out"], dtype=np.float64)
        total += o[0, 0]
        count += o[0, 1]
    if _trace:
        _cache["last_exec_time_ns"] = res.exec_time_ns
        _cache["last_results"] = res
    return np.float32(total / max(count, 1.0))
